# revision 55
# baseline (speedup 1.0000x reference)
"""Trainium2 Bass kernel for soft K-means assignment (vq_codebook).

reference computes, per sample row x_n (D=256) against K=512 centroids:
    dists[n,k] = ||x_n||^2 - 2 x_n.c_k + ||c_k||^2
    out[n,k]   = softmax_k(-dists[n,k] / T),  T = 0.1

softmax is invariant to per-row constants, so ||x||^2 drops out:
    out[n,:] = softmax_k(20 * (x.c_k - ||c_k||^2 / 2))

T=0.1 makes the softmax extremely sharp: near-tie rows need logits to
~16-bit input precision. That rules out one reduced-precision matmul
pass (fp32r measured ~12-bit effective on HW -> 6e-2 rel err, gate is
2e-2) but doesn't require fp32's 4-cycle-per-row exact path. The cross
term is computed as an exact fp16 product plus a cheap fp8 correction,
all at 1 cycle/row (or better) PE rates:

    x = x_h + x_l   (x_h = fp16(x); split on host - pure input
    c = c_h + c_l    marshaling, like the host-side transpose)
    x.c = x_h.c_h                      2 fp16 matmuls; 10-bit mantissa
                                       products are EXACT in fp32 PSUM
        + x_h.c_l + x_l.c_h            512-long contraction of fp8
                                       factors (scaled 2^+-6 into e4m3
                                       range) as 2 DoubleRow matmuls at
                                       0.5 cyc/row; the residual only
                                       needs ~1e-3 abs accuracy
        (+ x_l.c_l ~ 2^-22, dropped)
    - csq/2 enters PSUM via one 2-partition bf16 outer-product matmul
      per tile: rows = bf16-exact hi + bf16 lo remainder of the f32
      csq (computed on-chip), stacked on partitions 0/1 through a pair
      of 2x2-selector matmuls (engines cannot write partition 1).

numpy-validated max abs err of the scheme on the real data: 7.8e-3;
measured on hardware: 7.8e-3 (gate 2e-2).

Structure (8 cores, data-parallel over the flattened sample axis;
4096 rows/core in 32 tiles of 128 partitions x 512 clusters):
  - per tile: 5-matmul PSUM accumulation group (fp16 hh x2, fp8
    DoubleRow residual x2, bf16 bias x1), then elementwise work spread
    across engines so the ~360 B/ns DMA device stays the bottleneck:
      DVE:  mx = row max of u (PSUM), mxs = -20*mx, r = 1/s
      ACT:  e = exp(20*u + mxs) with accumulated row sum s
      Pool: o = e * r   (alternating tiles on DVE, which runs the
            SBUF-only multiply at 2 elem/cycle)
  - a burst of dummy bf16 matmuls at program start ramps the PE
    p-state to full speed while the first DMAs are in flight.
  - DMAs are batched (4 input tiles / 2 output tiles per instruction)
    to amortize the ~630ns HWDGE config cost; input loads are queued
    ahead of output stores on the SP queue because a DMA's sem waits
    hold the issuing sequencer (stores would head-of-line-block
    loads); first/last output batches go per-tile to trim ramp/drain.
  - pitfalls baked in from this and a previous session:
    tensor_tensor_reduce / scalar_tensor_tensor / negated reduce /
    ACT copy-with-scale-AP misbehave or crash; GPSIMD cannot access
    PSUM; engine APs cannot start at partition 1; fp32r matmul
    operands must be produced by an explicit rounding instruction.
"""

import numpy as np
import ml_dtypes
from contextlib import ExitStack

import concourse.bass as bass
import concourse.bacc as bacc
import concourse.mybir as mybir
import concourse.tile as tile
from concourse.bass_utils import run_bass_kernel_spmd

N_CORES = 8
B, S, D = 32, 1024, 256
K = 512
N_TOTAL = B * S              # 32768
N_PER_CORE = N_TOTAL // N_CORES  # 4096
P = 128                      # partitions / rows per tile
N_TILES = N_PER_CORE // P    # 32
TEMPERATURE = 0.1
SCALE = 2.0 / TEMPERATURE    # 20
RSCALE = 2.0 ** 6            # fp8 residual factor pre-scale

IN_BATCH = 4                 # row tiles per input DMA
OUT_BATCH = 2                # row tiles per output DMA

F32 = mybir.dt.float32
F16 = mybir.dt.float16
BF16 = mybir.dt.bfloat16
FP8 = mybir.dt.float8e4
NP_F16 = np.float16
NP_BF16 = ml_dtypes.bfloat16
NP_FP8 = ml_dtypes.float8_e4m3
WARMUP_MM = 8                # dummy matmuls to ramp the PE p-state
X8_SHIP = "full"             # "full": both fp8 factors from host;
                             # "lo": ship x_l/c_l only, derive hi on-chip


def build_program(mul_engine="alt", mxs_engine="vector", warmup=WARMUP_MM,
                  psum_bufs=6, x_bufs=8, e_bufs=4, o_bufs=4, bias_path="dma", conv_engine="vector",
                  in_batch=IN_BATCH, out_batch=OUT_BATCH, n_prefetch=3, first_single=0,
                  x8_ship="full", cship="full", early_dve=0, ksplit=1,
                  in_dma="sync", out_dma="sync"):
    nc = bacc.Bacc("TRN2", target_bir_lowering=False, debug=False)
    # All operands arrive HOST-PRE-TRANSPOSED with the contraction dim
    # d on partitions. x8/c8 ship only the LOW fp8 residual factors
    # (x_l*64 / c_l*64, 2 stacked 128-row d-chunks); the HIGH factors
    # (x_h/64, c_h/64) are derived on-chip from the fp16 tiles already
    # in SBUF with a DVE scale-convert, halving the fp8 input traffic.
    xh_in = nc.dram_tensor("xh", [D, N_PER_CORE], F16, kind="ExternalInput")
    x8_rows = 4 * P if x8_ship == "full" else 2 * P
    x8_in = nc.dram_tensor("x8", [x8_rows, N_PER_CORE], FP8,
                           kind="ExternalInput")
    ch_in = nc.dram_tensor("ch", [D, K], F16, kind="ExternalInput")
    c8_in = nc.dram_tensor("c8", [x8_rows, K], FP8, kind="ExternalInput")
    c_in = nc.dram_tensor("centroids", [D, K], F32, kind="ExternalInput")
    out = nc.dram_tensor("out", [N_PER_CORE, K], F32, kind="ExternalOutput")

    n_dchunks = D // P   # 2
    DR = mybir.MatmulPerfMode.DoubleRow

    with tile.TileContext(nc) as tc, ExitStack() as ctx:
        singles = ctx.enter_context(tc.tile_pool(name="singles", bufs=1))
        psum = ctx.enter_context(tc.tile_pool(name="psum", bufs=psum_bufs,
                                              space="PSUM"))

        # ---- PE p-state warm-up: dummy bf16 matmuls with no data deps ----
        if warmup:
            wsrc = singles.tile([P, K], BF16)
            nc.gpsimd.memset(wsrc[:], 1.0)
            wps = psum.tile([P, K], F32, tag="warm", bufs=1)
            for _ in range(warmup):
                nc.tensor.matmul(wps[:], wsrc[:, :P], wsrc[:],
                                 start=True, stop=True)

        # ---- centroid operands ----
        cT = singles.tile([P, n_dchunks, K], F32)    # f32, for csq only
        Kc = K // ksplit
        for kh in range(ksplit):     # split so the csq path starts early
            for j in range(n_dchunks):
                nc.sync.dma_start(
                    out=cT[:, j, kh * Kc:(kh + 1) * Kc],
                    in_=c_in.ap()[j * P:(j + 1) * P, kh * Kc:(kh + 1) * Kc])
        ch = singles.tile([P, n_dchunks, K], F16)
        if cship == "min":
            # derive all centroid operands from the single f32 ship:
            # ch = fp16(c); c_l = c - ch; c8 = [fp8(c_l*64); fp8(ch/64)]
            nc.vector.tensor_copy(ch[:], cT[:])
            c8 = singles.tile([P, 4, K], FP8)
            cl_f = singles.tile([P, n_dchunks, K], F32)
            nc.vector.tensor_tensor(out=cl_f[:], in0=cT[:], in1=ch[:],
                                    op=mybir.AluOpType.subtract)
            nc.vector.tensor_scalar_mul(c8[:, 0:2, :], cl_f[:], RSCALE)
            nc.vector.tensor_scalar_mul(c8[:, 2:4, :], ch[:], 1.0 / RSCALE)
        elif x8_ship == "full":
            nc.sync.dma_start(
                out=ch[:],
                in_=ch_in.ap().rearrange("(j p) k -> p j k", j=n_dchunks))
            c8 = singles.tile([P, 4, K], FP8)
            nc.sync.dma_start(
                out=c8[:],
                in_=c8_in.ap().rearrange("(s p) k -> p s k", s=4))
        else:
            nc.sync.dma_start(
                out=ch[:],
                in_=ch_in.ap().rearrange("(j p) k -> p j k", j=n_dchunks))
            c8 = singles.tile([P, 4, K], FP8)
            nc.sync.dma_start(
                out=c8[:, 0:2, :],
                in_=c8_in.ap().rearrange("(s p) k -> p s k", s=2))
            nc.vector.tensor_scalar_mul(c8[:, 2:4, :], ch[:], 1.0 / RSCALE)

        # ---- input loads: first `n_prefetch` batches queued up front on
        # SP; the rest are interleaved between output stores.
        xhpool = ctx.enter_context(tc.tile_pool(name="xh", bufs=x_bufs))
        x8pool = ctx.enter_context(tc.tile_pool(name="x8", bufs=x_bufs))
        n_in = in_batch * P      # rows per input DMA
        n_out = out_batch * P    # rows per output DMA
        n_batches = N_TILES // in_batch

        in_eng = nc.gpsimd if in_dma == "gpsimd" else nc.sync

        def load_x(b):
            sl = slice(b * n_in, (b + 1) * n_in)
            xh_sb = xhpool.tile([P, n_dchunks, n_in], F16, tag="xh")
            in_eng.dma_start(
                out=xh_sb[:],
                in_=xh_in.ap()[:, sl].rearrange("(j p) n -> p j n",
                                                j=n_dchunks))
            x8_sb = x8pool.tile([P, 4, n_in], FP8, tag="x8")
            if x8_ship == "full":
                in_eng.dma_start(
                    out=x8_sb[:],
                    in_=x8_in.ap()[:, sl].rearrange("(s p) n -> p s n", s=4))
            else:
                # ship x_l*64 only; derive x_h/64 on-chip, alternating
                # the convert between DVE and Pool to stay off the pace
                nc.sync.dma_start(
                    out=x8_sb[:, 2:4, :],
                    in_=x8_in.ap()[:, sl].rearrange("(s p) n -> p s n", s=2))
                conv = nc.vector if b % 2 == 0 else nc.gpsimd
                conv.tensor_scalar_mul(x8_sb[:, 0:2, :], xh_sb[:],
                                       1.0 / RSCALE)
            return xh_sb, x8_sb

        prefetch = min(n_prefetch, n_batches)
        x_tiles = {b: load_x(b) for b in range(prefetch)}

        # ---- setup: bias rows = +csq (sign folded into the -0.5 row),
        # split as bf16 hi (exact under bf16 matmul) + bf16 lo remainder.
        # csq = sum_d cT^2 lands DUPLICATED on partitions 0 and 1 via a
        # [128,2]-ones matmul (out partitions = lhsT free size), so the
        # bias hi/lo pair can be built with pure lane ops and feed ONE
        # 2-contraction bias matmul per tile.
        sq = singles.tile([P, n_dchunks, K], F32)
        csq_parts = singles.tile([1, n_dchunks, K], F32)
        csq2 = singles.tile([1, K], F32)
        hi_bf = singles.tile([1, K], BF16)
        hi_f = singles.tile([1, K], F32)
        lo_bf = singles.tile([1, K], BF16)
        for kh in range(ksplit):     # per-K-half pipeline behind the DMAs
            ks = slice(kh * Kc, (kh + 1) * Kc)
            for j in range(n_dchunks):   # j-split: square/reduce pipeline
                nc.scalar.square(sq[:, j, ks], cT[:, j, ks])
                nc.gpsimd.tensor_reduce(out=csq_parts[:, j, ks],
                                        in_=sq[:, j, ks],
                                        axis=mybir.AxisListType.C,
                                        op=mybir.AluOpType.add)
            nc.vector.tensor_tensor(out=csq2[:, ks],
                                    in0=csq_parts[:, 0, ks],
                                    in1=csq_parts[:, 1, ks],
                                    op=mybir.AluOpType.add)
            nc.vector.tensor_copy(hi_bf[:, ks], csq2[:, ks])
            nc.vector.tensor_copy(hi_f[:, ks], hi_bf[:, ks])
            nc.vector.tensor_tensor(out=lo_bf[:, ks], in0=csq2[:, ks],
                                    in1=hi_f[:, ks],
                                    op=mybir.AluOpType.subtract)

        if bias_path == "dma":
            # hi/lo stacked on partitions 0/1 -> ONE 2-contraction matmul
            # per tile. Engines cannot write partition 1 directly, so
            # route the rows through the PE with 2x2 selector weights
            # ([1,0] / [0,1] outer products land hi on p0, lo on p1),
            # then one legal [0:2] DVE copy back to SBUF bf16.
            neghalf_rows = singles.tile([2, P], BF16)
            nc.vector.memset(neghalf_rows[:], -0.5)
            wsel = singles.tile([1, 2, 2], BF16)
            nc.vector.memset(wsel[:], 0.0)
            nc.vector.memset(wsel[:, 0, 0:1], 1.0)
            nc.vector.memset(wsel[:, 1, 1:2], 1.0)
            pair_ps = psum.tile([2, K], F32, tag="pair", bufs=1)
            bias_pair = singles.tile([2, K], BF16)
            for kh in range(ksplit):
                ks = slice(kh * Kc, (kh + 1) * Kc)
                # start=True zeroes the whole 2KB PSUM region, so only
                # the first half may set it; halves run in PE order.
                nc.tensor.matmul(pair_ps[:, ks], wsel[:, 0, :],
                                 hi_bf[:, ks],
                                 start=(kh == 0), stop=False)
                nc.tensor.matmul(pair_ps[:, ks], wsel[:, 1, :],
                                 lo_bf[:, ks],
                                 start=False, stop=(kh == ksplit - 1))
                nc.vector.tensor_copy(bias_pair[:, ks], pair_ps[:, ks])
            bias_mms = [(neghalf_rows, bias_pair)]
        else:
            neghalf_row = singles.tile([1, P], BF16)
            nc.vector.memset(neghalf_row[:], -0.5)
            bias_mms = [(neghalf_row, hi_bf), (neghalf_row, lo_bf)]

        # ---- main loop over 128-row tiles ----
        epool = ctx.enter_context(tc.tile_pool(name="e", bufs=e_bufs))
        opool = ctx.enter_context(tc.tile_pool(name="o", bufs=o_bufs))
        stats = ctx.enter_context(tc.tile_pool(name="stats", bufs=8))

        for b in range(n_batches):
            xh_sb, x8_sb = x_tiles[b]
            for ob in range(in_batch // out_batch):
                o_sb = opool.tile([P, out_batch, K], F32, tag="o")
                for i in range(out_batch):
                    tt = ob * out_batch + i          # tile within batch
                    cols = slice(tt * P, (tt + 1) * P)
                    u_ps = psum.tile([P, K], F32, tag="u", bufs=psum_bufs)
                    # exact fp16 hi product
                    for j in range(n_dchunks):
                        nc.tensor.matmul(u_ps[:], xh_sb[:, j, cols],
                                         ch[:, j, :],
                                         start=(j == 0), stop=False)
                    # fp8 residual, 256-deep contraction per DoubleRow MM
                    for g in range(2):
                        nc.tensor.matmul(u_ps[:],
                                         x8_sb[:, 2 * g:2 * g + 2, cols],
                                         c8[:, 2 * g:2 * g + 2, :],
                                         perf_mode=DR,
                                         start=False, stop=False)
                    # -csq/2 bias rows
                    for bi, (lrow, rrow) in enumerate(bias_mms):
                        nc.tensor.matmul(u_ps[:], lrow[:], rrow[:],
                                         start=False,
                                         stop=(bi == len(bias_mms) - 1))

                    # row max of u (logits = 20*u), then e = exp(20u - 20mx)
                    mx = stats.tile([P, 1], F32, tag="mx")
                    nc.vector.tensor_reduce(out=mx[:], in_=u_ps[:],
                                            axis=mybir.AxisListType.X,
                                            op=mybir.AluOpType.max)
                    mxs = stats.tile([P, 1], F32, tag="mxs")
                    if mxs_engine == "gpsimd":
                        nc.gpsimd.tensor_scalar_mul(mxs[:], mx[:], -SCALE)
                    else:
                        nc.vector.tensor_scalar_mul(mxs[:], mx[:], -SCALE)

                    e_sb = epool.tile([P, K], F32, tag="e")
                    s_sb = stats.tile([P, 1], F32, tag="s")
                    nc.scalar.activation(e_sb[:], u_ps[:],
                                         mybir.ActivationFunctionType.Exp,
                                         bias=mxs[:], scale=SCALE,
                                         accum_out=s_sb[:])

                    r_sb = stats.tile([P, 1], F32, tag="r")
                    nc.vector.reciprocal(r_sb[:], s_sb[:])

                    t_global = b * in_batch + tt
                    # last tiles: mul on DVE (shorter latency) to trim
                    # the drain tail
                    use_pool = (mul_engine == "gpsimd" or
                                (mul_engine == "alt" and t_global % 2 == 0
                                 and t_global >= early_dve))
                    if use_pool and t_global < N_TILES - 2:
                        nc.gpsimd.tensor_scalar_mul(o_sb[:, i, :], e_sb[:],
                                                    r_sb[:])
                    else:
                        nc.vector.tensor_scalar_mul(o_sb[:, i, :], e_sb[:],
                                                    r_sb[:])

                row0 = b * n_in + ob * n_out
                out_eng = nc.gpsimd if out_dma == "gpsimd" else nc.sync
                last = (b == n_batches - 1 and ob == in_batch // out_batch - 1)
                if last or b <= first_single:
                    # split the final store per tile so the kernel's tail
                    # doesn't wait for the whole batch before draining
                    for i in range(out_batch):
                        r0 = row0 + i * P
                        out_eng.dma_start(out=out.ap()[r0:r0 + P, :],
                                          in_=o_sb[:, i, :])
                else:
                    out_eng.dma_start(
                        out=out.ap()[row0:row0 + n_out, :]
                            .rearrange("(c p) k -> p c k", c=out_batch),
                        in_=o_sb[:])
                if ob == 0 and b + prefetch < n_batches:
                    x_tiles[b + prefetch] = load_x(b + prefetch)

    nc.compile()
    return nc


def _split_inputs(x, centroids):
    """Host-side marshaling: transpose + fp16/fp8 precision split."""
    xf = np.asarray(x, dtype=np.float32).reshape(N_TOTAL, D)
    c = np.asarray(centroids, dtype=np.float32)

    def split(a):              # a: [n, D] f32 -> hi fp16 / fp8 factors, .T
        a64 = a.astype(np.float64)
        h = a.astype(NP_F16)
        l = a64 - h.astype(np.float64)
        hT = np.ascontiguousarray(h.T)                      # [D, n] fp16
        h8 = np.ascontiguousarray((a64 / RSCALE).astype(NP_FP8).T)
        l8 = np.ascontiguousarray((l * RSCALE).astype(NP_FP8).T)
        return hT, h8, l8                                   # [D, n] each

    xhT, xh8T, xl8T = split(xf)
    chT, ch8T, cl8T = split(c)
    if X8_SHIP == "full":
        # term-1 = (x_h/64).(c_l*64); term-2 = (x_l*64).(c_h/64)
        x8 = np.ascontiguousarray(np.concatenate([xh8T, xl8T], axis=0))
        c8 = np.ascontiguousarray(np.concatenate([cl8T, ch8T], axis=0))
    else:
        x8 = xl8T
        c8 = cl8T
    cT = np.ascontiguousarray(c.T)
    return xhT, x8, chT, c8, cT


_CACHED_NC = None


def kernel(x, centroids):
    global _CACHED_NC
    if _CACHED_NC is None:
        _CACHED_NC = build_program(x8_ship=X8_SHIP)
    nc = _CACHED_NC

    xhT, x8, chT, c8, cT = _split_inputs(x, centroids)
    in_maps = []
    for i in range(N_CORES):
        sl = slice(i * N_PER_CORE, (i + 1) * N_PER_CORE)
        in_maps.append({
            "xh": np.ascontiguousarray(xhT[:, sl]),
            "x8": np.ascontiguousarray(x8[:, sl]),
            "ch": chT,
            "c8": c8,
            "centroids": cT,
        })
    res = run_bass_kernel_spmd(nc, in_maps, core_ids=list(range(N_CORES)))
    outs = np.concatenate([r["out"] for r in res.results], axis=0)
    return outs.reshape(B, S, K)
